# revision 5
# baseline (speedup 1.0000x reference)
"""CascadeRCNN head on 8 TRN2 cores — hybrid sharding.

Stages 0/1: data-parallel over rois (each core runs its own 125 rois through
the full fc1 in fp8 DoubleRow, streaming the full weight per jy-row batch) —
no collectives at all.
Ensemble (3 heads on the final pooled features): K-sharded by ROIAlign jy-row
(core c owns row c; core 7 is a zero-weight dummy), partial h1 summed by one
ReduceScatter; one AllGather distributes the final-stage rois first. Since a
core's ensemble units share one jy, the y-lerp scale is a single per-roi
scalar. All GEMMs fp8-DR; gathers/interp bf16; roi geometry fp32.
"""

import numpy as np
from contextlib import ExitStack

import concourse.bass as bass
import concourse.tile as tile
from concourse import bacc, mybir
from concourse.masks import make_identity

F32 = mybir.dt.float32
BF16 = mybir.dt.bfloat16
F8 = mybir.dt.float8e4
I32 = mybir.dt.int32
Alu = mybir.AluOpType
Act = mybir.ActivationFunctionType
DR = mybir.MatmulPerfMode.DoubleRow

N_CORES = 8
R = 125              # rois per core / chunk
NCH = 8              # chunks (ensemble)
POOL = 7
C = 256
NU = 7               # positions per jy-row batch / ensemble units per core
KB = 2 * NU          # 14 k-blocks of 128 per batch
HID = 1024
NCLS = 81
IMG = 1024.0

FEAT_ROWS = 256 * 256 + 128 * 128 + 64 * 64 + 32 * 32  # 87040
GROUPS = [[i for i in range(N_CORES)]]


# --------------------------------------------------------------------------
# roi prep — stage (single chunk, all 49 positions)
# --------------------------------------------------------------------------

def _roi_prep_stage(nc, pools, rois_my, grid_t):
    """rois_my [R,4] -> idx [R, 7jy, 7jx, 2] i32 (x256), wx [R,7], wy [R,7]."""
    v = nc.vector
    prep = pools["prep"]

    def pt(cols, dtype=F32, tag=None):
        return prep.tile([R, cols], dtype, tag=tag, name=tag)

    y1 = rois_my[:, 0:1]; x1 = rois_my[:, 1:2]
    y2 = rois_my[:, 2:3]; x2 = rois_my[:, 3:4]

    hh = pt(1, tag="hh"); v.tensor_tensor(hh[:], y2, y1, op=Alu.subtract)
    ww = pt(1, tag="ww"); v.tensor_tensor(ww[:], x2, x1, op=Alu.subtract)
    hw = pt(1, tag="hw"); v.tensor_tensor(hw[:], hh[:], ww[:], op=Alu.mult)
    v.tensor_scalar(hw[:], hw[:], 1e-6, None, op0=Alu.max)

    g2 = pt(1, tag="g2"); v.tensor_scalar(g2[:], hw[:], 12544.0, None, op0=Alu.is_ge)
    g3 = pt(1, tag="g3"); v.tensor_scalar(g3[:], hw[:], 50176.0, None, op0=Alu.is_ge)
    g4 = pt(1, tag="g4"); v.tensor_scalar(g4[:], hw[:], 200704.0, None, op0=Alu.is_ge)

    invs = pt(1, tag="invs")
    v.tensor_scalar(invs[:], g2[:], -0.125, 0.25, op0=Alu.mult, op1=Alu.add)
    t0 = pt(1, tag="t0")
    v.tensor_scalar(t0[:], g3[:], -0.0625, None, op0=Alu.mult)
    v.tensor_tensor(invs[:], invs[:], t0[:], op=Alu.add)
    v.tensor_scalar(t0[:], g4[:], -0.03125, None, op0=Alu.mult)
    v.tensor_tensor(invs[:], invs[:], t0[:], op=Alu.add)

    S = pt(1, tag="S"); v.tensor_scalar(S[:], invs[:], 1024.0, None, op0=Alu.mult)
    base = pt(1, tag="base")
    v.tensor_scalar(base[:], g2[:], 65536.0, None, op0=Alu.mult)
    v.tensor_scalar(t0[:], g3[:], 16384.0, None, op0=Alu.mult)
    v.tensor_tensor(base[:], base[:], t0[:], op=Alu.add)
    v.tensor_scalar(t0[:], g4[:], 4096.0, None, op0=Alu.mult)
    v.tensor_tensor(base[:], base[:], t0[:], op=Alu.add)
    Sm1 = pt(1, tag="Sm1"); v.tensor_scalar(Sm1[:], S[:], -1.0, None, op0=Alu.add)
    Sm2 = pt(1, tag="Sm2"); v.tensor_scalar(Sm2[:], S[:], -2.0, None, op0=Alu.add)

    sy1 = pt(1, tag="sy1"); v.tensor_tensor(sy1[:], y1, invs[:], op=Alu.mult)
    sx1 = pt(1, tag="sx1"); v.tensor_tensor(sx1[:], x1, invs[:], op=Alu.mult)
    sy2 = pt(1, tag="sy2"); v.tensor_tensor(sy2[:], y2, invs[:], op=Alu.mult)
    sx2 = pt(1, tag="sx2"); v.tensor_tensor(sx2[:], x2, invs[:], op=Alu.mult)
    dy = pt(1, tag="dy"); v.tensor_tensor(dy[:], sy2[:], sy1[:], op=Alu.subtract)
    dx = pt(1, tag="dx"); v.tensor_tensor(dx[:], sx2[:], sx1[:], op=Alu.subtract)

    def axis_prep(scoord, dcoord, suffix):
        ys = pt(POOL, tag="ys" + suffix)
        v.tensor_scalar(ys[:], grid_t[0:R, :], dcoord[:], scoord[:],
                        op0=Alu.mult, op1=Alu.add)
        yi = pt(POOL, I32, tag="yi" + suffix)
        v.tensor_copy(yi[:], ys[:])
        yf = pt(POOL, tag="yf" + suffix)
        v.tensor_copy(yf[:], yi[:])
        gt = pt(POOL, tag="gt" + suffix)
        v.tensor_tensor(gt[:], yf[:], ys[:], op=Alu.is_gt)
        y0f = pt(POOL, tag="y0f" + suffix)
        v.tensor_tensor(y0f[:], yf[:], gt[:], op=Alu.subtract)
        y0c = pt(POOL, tag="y0c" + suffix)
        v.tensor_scalar(y0c[:], y0f[:], Sm1[:], None, op0=Alu.min)
        by = pt(POOL, tag="by" + suffix)
        v.tensor_scalar(by[:], y0c[:], Sm2[:], None, op0=Alu.min)
        wy = pt(POOL, tag="wy" + suffix)
        v.tensor_tensor(wy[:], ys[:], y0c[:], op=Alu.subtract)
        v.tensor_scalar(wy[:], wy[:], 0.0, 1.0, op0=Alu.max, op1=Alu.min)
        fl = pt(POOL, tag="fl" + suffix)
        v.tensor_scalar(fl[:], y0f[:], Sm1[:], None, op0=Alu.is_ge)
        v.tensor_tensor(wy[:], wy[:], fl[:], op=Alu.max)
        return by, wy

    by, wy = axis_prep(sy1, dy, "Y")
    bx, wx = axis_prep(sx1, dx, "X")

    # idx[r, jy, jx, yn] = (base + by*S + bx (+S)) * 256
    idxf = pools["idx"].tile([R, POOL, POOL, 2], F32, tag="idxf", name="idxf")
    rowA = pt(POOL, tag="rowA")
    v.tensor_scalar(rowA[:], by[:], S[:], base[:], op0=Alu.mult, op1=Alu.add)
    v.tensor_tensor(idxf[:, :, :, 0],
                    rowA[:, :, None].to_broadcast([R, POOL, POOL]),
                    bx[:, None, :].to_broadcast([R, POOL, POOL]), op=Alu.add)
    v.tensor_scalar(idxf[:, :, :, 1], idxf[:, :, :, 0], S[:], None, op0=Alu.add)
    idx_i32 = pools["idx"].tile([R, POOL, POOL, 2], I32, tag="idxi", name="idxi")
    v.tensor_copy(idx_i32[:], idxf[:])
    v.tensor_scalar(idx_i32[:], idx_i32[:], C, None, op0=Alu.mult)
    return idx_i32, wx, wy


# --------------------------------------------------------------------------
# roi prep — ensemble (8 chunks, per-core jy-row; y collapses to one scalar)
# --------------------------------------------------------------------------

def _roi_prep_ens(nc, pools, rois_t, gy_t, grid_t):
    """rois_t [R,8,4] -> idx [R,8,NU,2] i32 (x256), wx [R,8,NU], wy [R,8,1]."""
    v = nc.vector
    prep = pools["prep"]

    def pt(shape, dtype=F32, tag=None):
        return prep.tile(shape, dtype, tag=tag, name=tag)

    def bc(ap, last=NU):
        return ap[:, :, None].to_broadcast([R, NCH, last])

    y1 = rois_t[:, :, 0]; x1 = rois_t[:, :, 1]
    y2 = rois_t[:, :, 2]; x2 = rois_t[:, :, 3]

    hh = pt([R, NCH], tag="ehh"); v.tensor_tensor(hh[:], y2, y1, op=Alu.subtract)
    ww = pt([R, NCH], tag="eww"); v.tensor_tensor(ww[:], x2, x1, op=Alu.subtract)
    hw = pt([R, NCH], tag="ehw"); v.tensor_tensor(hw[:], hh[:], ww[:], op=Alu.mult)
    v.tensor_scalar(hw[:], hw[:], 1e-6, None, op0=Alu.max)

    g2 = pt([R, NCH], tag="eg2"); v.tensor_scalar(g2[:], hw[:], 12544.0, None, op0=Alu.is_ge)
    g3 = pt([R, NCH], tag="eg3"); v.tensor_scalar(g3[:], hw[:], 50176.0, None, op0=Alu.is_ge)
    g4 = pt([R, NCH], tag="eg4"); v.tensor_scalar(g4[:], hw[:], 200704.0, None, op0=Alu.is_ge)

    invs = pt([R, NCH], tag="einvs")
    v.tensor_scalar(invs[:], g2[:], -0.125, 0.25, op0=Alu.mult, op1=Alu.add)
    t0 = pt([R, NCH], tag="et0")
    v.tensor_scalar(t0[:], g3[:], -0.0625, None, op0=Alu.mult)
    v.tensor_tensor(invs[:], invs[:], t0[:], op=Alu.add)
    v.tensor_scalar(t0[:], g4[:], -0.03125, None, op0=Alu.mult)
    v.tensor_tensor(invs[:], invs[:], t0[:], op=Alu.add)

    S = pt([R, NCH], tag="eS"); v.tensor_scalar(S[:], invs[:], 1024.0, None, op0=Alu.mult)
    base = pt([R, NCH], tag="ebase")
    v.tensor_scalar(base[:], g2[:], 65536.0, None, op0=Alu.mult)
    v.tensor_scalar(t0[:], g3[:], 16384.0, None, op0=Alu.mult)
    v.tensor_tensor(base[:], base[:], t0[:], op=Alu.add)
    v.tensor_scalar(t0[:], g4[:], 4096.0, None, op0=Alu.mult)
    v.tensor_tensor(base[:], base[:], t0[:], op=Alu.add)
    Sm1 = pt([R, NCH], tag="eSm1"); v.tensor_scalar(Sm1[:], S[:], -1.0, None, op0=Alu.add)
    Sm2 = pt([R, NCH], tag="eSm2"); v.tensor_scalar(Sm2[:], S[:], -2.0, None, op0=Alu.add)

    sy1 = pt([R, NCH], tag="esy1"); v.tensor_tensor(sy1[:], y1, invs[:], op=Alu.mult)
    sx1 = pt([R, NCH], tag="esx1"); v.tensor_tensor(sx1[:], x1, invs[:], op=Alu.mult)
    sy2 = pt([R, NCH], tag="esy2"); v.tensor_tensor(sy2[:], y2, invs[:], op=Alu.mult)
    sx2 = pt([R, NCH], tag="esx2"); v.tensor_tensor(sx2[:], x2, invs[:], op=Alu.mult)
    dy = pt([R, NCH], tag="edy"); v.tensor_tensor(dy[:], sy2[:], sy1[:], op=Alu.subtract)
    dx = pt([R, NCH], tag="edx"); v.tensor_tensor(dx[:], sx2[:], sx1[:], op=Alu.subtract)

    def axis_prep(scoord, dcoord, grid_ap, n, suffix):
        ys = pt([R, NCH, n], tag="eys" + suffix)
        v.tensor_tensor(ys[:], grid_ap, bc(dcoord, n), op=Alu.mult)
        v.tensor_tensor(ys[:], ys[:], bc(scoord, n), op=Alu.add)
        yi = pt([R, NCH, n], I32, tag="eyi" + suffix)
        v.tensor_copy(yi[:], ys[:])
        yf = pt([R, NCH, n], tag="eyf" + suffix)
        v.tensor_copy(yf[:], yi[:])
        gt = pt([R, NCH, n], tag="egt" + suffix)
        v.tensor_tensor(gt[:], yf[:], ys[:], op=Alu.is_gt)
        y0f = pt([R, NCH, n], tag="ey0f" + suffix)
        v.tensor_tensor(y0f[:], yf[:], gt[:], op=Alu.subtract)
        y0c = pt([R, NCH, n], tag="ey0c" + suffix)
        v.tensor_tensor(y0c[:], y0f[:], bc(Sm1, n), op=Alu.min)
        by = pt([R, NCH, n], tag="eby" + suffix)
        v.tensor_tensor(by[:], y0c[:], bc(Sm2, n), op=Alu.min)
        wy = pt([R, NCH, n], tag="ewy" + suffix)
        v.tensor_tensor(wy[:], ys[:], y0c[:], op=Alu.subtract)
        v.tensor_scalar(wy[:], wy[:], 0.0, 1.0, op0=Alu.max, op1=Alu.min)
        fl = pt([R, NCH, n], tag="efl" + suffix)
        v.tensor_tensor(fl[:], y0f[:], bc(Sm1, n), op=Alu.is_ge)
        v.tensor_tensor(wy[:], wy[:], fl[:], op=Alu.max)
        return by, wy

    gyb = gy_t[0:R, None, 0:1].to_broadcast([R, NCH, 1])
    gxb = grid_t[0:R, None, :].to_broadcast([R, NCH, NU])
    by, wy = axis_prep(sy1, dy, gyb, 1, "Y")
    bx, wx = axis_prep(sx1, dx, gxb, NU, "X")

    idxf = pools["idx"].tile([R, NCH, NU, 2], F32, tag="idxf", name="idxf")
    rowA = pt([R, NCH, 1], tag="erowA")
    v.tensor_tensor(rowA[:], by[:], S[:, :, None], op=Alu.mult)
    v.tensor_tensor(rowA[:], rowA[:], base[:, :, None], op=Alu.add)
    v.tensor_tensor(idxf[:, :, :, 0],
                    rowA[:].to_broadcast([R, NCH, NU]), bx[:], op=Alu.add)
    v.tensor_tensor(idxf[:, :, :, 1], idxf[:, :, :, 0], bc(S), op=Alu.add)
    idx_i32 = pools["idx"].tile([R, NCH, NU, 2], I32, tag="idxi", name="idxi")
    v.tensor_copy(idx_i32[:], idxf[:])
    v.tensor_scalar(idx_i32[:], idx_i32[:], C, None, op0=Alu.mult)
    return idx_i32, wx, wy


# --------------------------------------------------------------------------
# pooling batch: gather + bilinear + transpose -> xt (fp8)
# --------------------------------------------------------------------------

def _pool_batch(nc, pools, feats_ap, idx_ap, wy_scalar, wx_fn, ident_b):
    """One 7-position batch with a shared jy: idx_ap [R, 14] i32 (flat elem
    index), wy_scalar [R,1]-ish AP, wx_fn(u) -> [R,1]-ish AP.
    Returns xt tile [128, KB*128] fp8 (x^T for the fc1 GEMM)."""
    v = nc.vector
    G = pools["gath"].tile([R, NU, 2, 512], BF16, tag="G", name="G")
    nc.gpsimd.indirect_dma_start(
        out=G[:].rearrange("p u y e -> p (u y e)"), out_offset=None,
        in_=feats_ap[:].rearrange("(a r) c -> a (r c)", a=1),
        in_offset=bass.IndirectOffsetOnAxis(ap=idx_ap, axis=1),
    )
    # y-interp (single scalar per roi): G1 <- G1-G0 ; G1 *= wy ; G0 += G1
    v.tensor_tensor(G[:, :, 1, :], G[:, :, 1, :], G[:, :, 0, :], op=Alu.subtract)
    v.tensor_scalar(G[:, :, 1, :], G[:, :, 1, :], wy_scalar, None, op0=Alu.mult)
    v.tensor_tensor(G[:, :, 0, :], G[:, :, 0, :], G[:, :, 1, :], op=Alu.add)
    # x-interp: P = T0 + wx*(T1-T0)
    P = pools["interp"].tile([R, NU, C], BF16, tag="P", name="P")
    t0v = G[:, :, 0, 0:C]
    v.tensor_tensor(P[:], G[:, :, 0, C:2 * C], t0v, op=Alu.subtract)
    for u in range(NU):
        nc.scalar.mul(P[:, u, :], P[:, u, :], wx_fn(u))
    v.tensor_tensor(P[:], P[:], t0v, op=Alu.add)
    # transpose KB chan-blocks into xt (fp8 for the DoubleRow GEMM)
    xt = pools["xt"].tile([128, KB * 128], F8, tag="xt", name="xt")
    Pf = P[:].rearrange("p u c -> p (u c)")
    for k in range(KB):
        ps = pools["pt"].tile([128, R], BF16, space="PSUM", tag="ptr", name="ptr")
        nc.tensor.transpose(out=ps[:], in_=Pf[:, k * 128:(k + 1) * 128],
                            identity=ident_b[0:R, 0:R])
        if k % 2 == 0:
            nc.scalar.copy(xt[:, k * 128:k * 128 + R], ps[:])
        else:
            nc.vector.tensor_copy(xt[:, k * 128:k * 128 + R], ps[:])
    return xt


def _dr_matmuls(nc, psums, xt, w1_ap, start):
    """fp8 DoubleRow matmuls: 7 K pair-blocks of xt against w1_ap [128,KB,HID]."""
    for k2 in range(KB // 2):
        lhsT = xt[:, k2 * 256:(k2 + 1) * 256].rearrange(
            "p (i m) -> p i m", i=2)[:, :, 0:R]
        for j in range(2):
            nc.tensor.matmul(psums[j][:], lhsT=lhsT,
                             rhs=w1_ap[:, 2 * k2:2 * k2 + 2,
                                       j * 512:(j + 1) * 512],
                             start=(start and k2 == 0), stop=False,
                             perf_mode=DR)


def _bias_stop(nc, psums, b_ap, ones_b):
    for j in range(2):
        nc.tensor.matmul(psums[j][:], lhsT=ones_b[0:1, 0:R],
                         rhs=b_ap[0:1, j * 512:(j + 1) * 512],
                         start=False, stop=True)


def _fc_big(nc, pools, hT, rhs_fn, b_t, head, relu, ones_b):
    psums = [pools["pfc"].tile([R, 512], F32, space="PSUM", tag=f"pf{j}",
                               name=f"pf{j}") for j in range(2)]
    for k in range(HID // 128):
        lhsT = hT[:, k * 128:k * 128 + R]
        for j in range(2):
            nc.tensor.matmul(psums[j][:], lhsT=lhsT, rhs=rhs_fn(k, j * 512, 512),
                             start=(k == 0), stop=False)
    for j in range(2):
        nc.tensor.matmul(psums[j][:], lhsT=ones_b[0:1, 0:R],
                         rhs=b_t[0:1, head, j * 512:(j + 1) * 512],
                         start=False, stop=True)
    h = pools["h"].tile([R, HID], BF16, tag="hbig", name="hbig")
    if relu:
        nc.scalar.activation(h[:, 0:512], psums[0][:], Act.Relu)
        nc.vector.tensor_scalar(h[:, 512:1024], psums[1][:], 0.0, None,
                                op0=Alu.max)
    else:
        nc.scalar.copy(h[:, 0:512], psums[0][:])
        nc.vector.tensor_copy(h[:, 512:1024], psums[1][:])
    return h


def _fc_small(nc, pools, hT, w_t, b_t, head, n_out, ones_b):
    ps_full = pools["psm"].tile([R, NCLS], F32, space="PSUM", tag="pssm",
                                name="pssm")
    ps = ps_full[:, 0:n_out]
    for k in range(HID // 128):
        nc.tensor.matmul(ps, lhsT=hT[:, k * 128:k * 128 + R],
                         rhs=w_t[:, head, k, 0:n_out],
                         start=(k == 0), stop=False)
    nc.tensor.matmul(ps, lhsT=ones_b[0:1, 0:R], rhs=b_t[0:1, head, 0:n_out],
                     start=False, stop=True)
    out = pools["h"].tile([R, n_out], F32, tag=f"hsm{n_out}", name="hsm")
    nc.vector.tensor_copy(out[:], ps)
    return out


def _transpose_h(nc, pools, h, ident_b):
    hT = pools["ht"].tile([128, (HID // 128) * 128], BF16, tag="hT", name="hT")
    for t in range(HID // 128):
        ps = pools["pt"].tile([128, R], BF16, space="PSUM", tag="ptr", name="ptr")
        nc.tensor.transpose(out=ps[:], in_=h[:, t * 128:(t + 1) * 128],
                            identity=ident_b[0:R, 0:R])
        if t % 2 == 0:
            nc.scalar.copy(hT[:, t * 128:t * 128 + R], ps[:])
        else:
            nc.vector.tensor_copy(hT[:, t * 128:t * 128 + R], ps[:])
    return hT


def _softmax(nc, pools, logits, tag):
    v = nc.vector
    rmax = pools["prep"].tile([R, 1], F32, tag="rmax" + tag, name="rmax")
    v.tensor_reduce(rmax[:], logits[:], axis=mybir.AxisListType.X, op=Alu.max)
    nmax = pools["prep"].tile([R, 1], F32, tag="nmax" + tag, name="nmax")
    v.tensor_scalar(nmax[:], rmax[:], -1.0, None, op0=Alu.mult)
    e = pools["h"].tile([R, NCLS], F32, tag="smx" + tag, name="smx")
    nc.scalar.activation(e[:], logits[:], Act.Exp, bias=nmax[:], scale=1.0)
    ssum = pools["prep"].tile([R, 1], F32, tag="ssum" + tag, name="ssum")
    v.tensor_reduce(ssum[:], e[:], axis=mybir.AxisListType.X, op=Alu.add)
    rsum = pools["prep"].tile([R, 1], F32, tag="rsum" + tag, name="rsum")
    v.reciprocal(rsum[:], ssum[:])
    v.tensor_scalar(e[:], e[:], rsum[:], None, op0=Alu.mult)
    return e


def _delta2bbox(nc, pools, rois_my, deltas, stds_t):
    v = nc.vector
    prep = pools["prep"]

    def pt(tag):
        return prep.tile([R, 1], F32, tag=tag, name=tag)

    d = prep.tile([R, 4], F32, tag="dsc", name="dsc")
    v.tensor_tensor(d[:], deltas[:], stds_t[0:R, :], op=Alu.mult)
    y1 = rois_my[:, 0:1]; x1 = rois_my[:, 1:2]
    y2 = rois_my[:, 2:3]; x2 = rois_my[:, 3:4]
    hh = pt("b_h"); v.tensor_tensor(hh[:], y2, y1, op=Alu.subtract)
    ww = pt("b_w"); v.tensor_tensor(ww[:], x2, x1, op=Alu.subtract)
    hh2 = pt("b_h2"); v.tensor_scalar(hh2[:], hh[:], 0.5, None, op0=Alu.mult)
    cy = pt("b_cy"); v.tensor_tensor(cy[:], y1, hh2[:], op=Alu.add)
    t = pt("b_t"); v.tensor_tensor(t[:], d[:, 0:1], hh[:], op=Alu.mult)
    v.tensor_tensor(cy[:], cy[:], t[:], op=Alu.add)
    ww2 = pt("b_w2"); v.tensor_scalar(ww2[:], ww[:], 0.5, None, op0=Alu.mult)
    cx = pt("b_cx"); v.tensor_tensor(cx[:], x1, ww2[:], op=Alu.add)
    v.tensor_tensor(t[:], d[:, 1:2], ww[:], op=Alu.mult)
    v.tensor_tensor(cx[:], cx[:], t[:], op=Alu.add)
    eh = pt("b_eh"); nc.scalar.activation(eh[:], d[:, 2:3], Act.Exp)
    ew = pt("b_ew"); nc.scalar.activation(ew[:], d[:, 3:4], Act.Exp)
    v.tensor_tensor(hh[:], hh[:], eh[:], op=Alu.mult)
    v.tensor_tensor(ww[:], ww[:], ew[:], op=Alu.mult)
    v.tensor_scalar(hh2[:], hh[:], 0.5, None, op0=Alu.mult)
    v.tensor_scalar(ww2[:], ww[:], 0.5, None, op0=Alu.mult)
    rn = pools["rois"].tile([R, 4], F32, tag="rnew", name="rnew")
    v.tensor_tensor(rn[:, 0:1], cy[:], hh2[:], op=Alu.subtract)
    v.tensor_tensor(rn[:, 1:2], cx[:], ww2[:], op=Alu.subtract)
    v.tensor_tensor(rn[:, 2:3], cy[:], hh2[:], op=Alu.add)
    v.tensor_tensor(rn[:, 3:4], cx[:], ww2[:], op=Alu.add)
    v.tensor_scalar(rn[:], rn[:], 0.0, IMG, op0=Alu.max, op1=Alu.min)
    return rn


# --------------------------------------------------------------------------
# kernel body
# --------------------------------------------------------------------------

def build_kernel(ctx: ExitStack, tc: "tile.TileContext", aps: dict, rep: int):
    nc = tc.nc
    pools = {}
    for name, bufs, space in [
        ("const", 1, "SBUF"), ("rois", 2, "SBUF"), ("prep", 2, "SBUF"),
        ("idx", 1, "SBUF"), ("gath", 2, "SBUF"), ("interp", 2, "SBUF"),
        ("xt", 2, "SBUF"), ("w1s", 2, "SBUF"), ("w1e", 1, "SBUF"),
        ("w2", 1, "SBUF"), ("wsm", 1, "SBUF"), ("bias", 1, "SBUF"),
        ("h", 2, "SBUF"), ("ht", 2, "SBUF"), ("rsio", 2, "SBUF"),
        ("pt", 2, "PSUM"), ("pfc", 2, "PSUM"), ("psm", 1, "PSUM"),
    ]:
        pools[name] = ctx.enter_context(tc.tile_pool(name=name, bufs=bufs,
                                                     space=space))

    ident = pools["const"].tile([128, 128], F32, tag="ident", name="ident")
    make_identity(nc, ident[:])
    ident_b = pools["const"].tile([128, 128], BF16, tag="identb", name="identb")
    nc.vector.tensor_copy(ident_b[:], ident[:])
    ones_b = pools["const"].tile([1, 128], BF16, tag="onesb", name="onesb")
    nc.vector.memset(ones_b[:], 1.0)
    eighth_b = pools["const"].tile([1, 128], BF16, tag="eighthb", name="eighthb")
    nc.vector.memset(eighth_b[:], 0.125)
    grid_t = pools["const"].tile([128, POOL], F32, tag="grid", name="grid")
    nc.sync.dma_start(grid_t[:], aps["gridall"][:])
    gy_t = pools["const"].tile([128, 1], F32, tag="gy", name="gy")
    nc.sync.dma_start(gy_t[:], aps["gridy_e"][:])
    stds_t = pools["const"].tile([128, 4], F32, tag="stds", name="stds")
    nc.sync.dma_start(stds_t[:], aps["stds_c"][:])

    w2h = {}

    def get_w2(head, gate=None):
        if head not in w2h:
            t = pools["w2"].tile([128, HID // 128, HID], BF16,
                                 tag=f"w2{head}", name=f"w2{head}")
            if gate is not None:
                # tiny dummy write dependent on `gate` keeps the scheduler
                # from hoisting this load into an earlier phase's DMA pipe
                nc.gpsimd.tensor_copy(t[0:1, 0:1, 0:4], gate)
            nc.scalar.dma_start(
                t[:], aps["fc2_w"][:].rearrange("h (k p) n -> h p k n",
                                                p=128)[head])
            w2h[head] = t
        return w2h[head]
    get_w2(0)
    rois_my = pools["rois"].tile([R, 4], F32, tag="rmy", name="rmy")
    nc.sync.dma_start(rois_my[:], aps["rois_my"][:])
    rois_my = rois_my[:]

    cls_t = pools["wsm"].tile([128, 3, HID // 128, NCLS], BF16, tag="clsw",
                              name="clsw")
    nc.sync.dma_start(cls_t[:],
                      aps["cls_w"][:].rearrange("h (k p) n -> p h k n", p=128))
    reg_t = pools["wsm"].tile([128, 3, HID // 128, 4], BF16, tag="regw",
                              name="regw")
    nc.sync.dma_start(reg_t[:],
                      aps["reg_w"][:].rearrange("h (k p) n -> p h k n", p=128))
    b1_t = pools["bias"].tile([1, 3, HID], BF16, tag="b1", name="b1")
    nc.scalar.dma_start(b1_t[:], aps["fc1_b"][:])
    b2_t = pools["bias"].tile([1, 3, HID], BF16, tag="b2", name="b2")
    nc.scalar.dma_start(b2_t[:], aps["fc2_b"][:])
    bcls_t = pools["bias"].tile([1, 3, NCLS], BF16, tag="bcls", name="bcls")
    nc.scalar.dma_start(bcls_t[:], aps["cls_b"][:])
    breg_t = pools["bias"].tile([1, 3, 4], BF16, tag="breg", name="breg")
    nc.scalar.dma_start(breg_t[:], aps["reg_b"][:])

    feats_ap = aps["feats_b"]

    def head_tail(hs_h1, head):
        h1 = pools["h"].tile([R, HID], BF16, tag="h1", name="h1")
        nc.gpsimd.tensor_scalar(h1[:, 0:512], hs_h1[:, 0:512], 0.0, None,
                                op0=Alu.max)
        nc.scalar.activation(h1[:, 512:1024], hs_h1[:, 512:1024], Act.Relu)
        h1T = _transpose_h(nc, pools, h1, ident_b)
        w2_t = get_w2(head)
        h2 = _fc_big(nc, pools, h1T,
                     lambda k, o, sz: w2_t[:, k, o:o + sz],
                     b2_t, head, True, ones_b)
        return _transpose_h(nc, pools, h2, ident_b)

    # ---------------- stages 0/1: data-parallel, no collectives ----------
    for s in range(2):
        idx_i32, wx, wy = _roi_prep_stage(nc, pools, rois_my, grid_t)
        if s > 0:
            get_w2(s, gate=idx_i32[0:1, 0, 0:2, 0:2].rearrange('p a b -> p (a b)'))
        psums = [pools["pfc"].tile([R, 512], F32, space="PSUM", tag=f"pf{j}",
                                   name=f"pf{j}") for j in range(2)]
        for b in range(POOL):
            w1s = pools["w1s"].tile([128, KB, HID], F8, tag="w1s", name="w1s")
            nc.sync.dma_start(
                w1s[:], aps["w1f8"][:].rearrange(
                    "h (b k p) n -> h b p k n", b=POOL, p=128)[s, b])
            xt = _pool_batch(nc, pools, feats_ap,
                             idx_i32[:, b, :, :].rearrange("p u y -> p (u y)"),
                             wy[:, b:b + 1],
                             lambda u: wx[:, u:u + 1], ident_b)
            _dr_matmuls(nc, psums, xt, w1s[:], start=(b == 0))
        _bias_stop(nc, psums, b1_t[0:1, s, :], ones_b)
        h1 = pools["h"].tile([R, HID], BF16, tag="h1", name="h1")
        nc.scalar.activation(h1[:, 0:512], psums[0][:], Act.Relu)
        nc.vector.tensor_scalar(h1[:, 512:1024], psums[1][:], 0.0, None,
                                op0=Alu.max)
        h1T = _transpose_h(nc, pools, h1, ident_b)
        w2s_t = get_w2(s)
        h2 = _fc_big(nc, pools, h1T,
                     lambda k, o, sz: w2s_t[:, k, o:o + sz],
                     b2_t, s, True, ones_b)
        h2T = _transpose_h(nc, pools, h2, ident_b)
        deltas = _fc_small(nc, pools, h2T, reg_t, breg_t, s, 4, ones_b)
        rn = _delta2bbox(nc, pools, rois_my, deltas, stds_t)
        rois_my = rn[:]

    # ---------------- rois AllGather + K-sharded ensemble -----------------
    ag_in = nc.dram_tensor(f"ag_in_{rep}", [R, 4], F32)
    ag_out = nc.dram_tensor(f"ag_out_{rep}", [NCH, R, 4], F32,
                            addr_space="Shared")
    nc.sync.dma_start(ag_in[:], rois_my)
    nc.gpsimd.collective_compute(
        "AllGather", Alu.bypass, replica_groups=GROUPS,
        ins=[ag_in[:].opt()], outs=[ag_out[:].opt()])
    rois_t = pools["rois"].tile([R, NCH, 4], F32, tag="rall", name="rall")
    nc.sync.dma_start(rois_t[:], ag_out[:].rearrange("c p d -> p c d"))

    rs_e_in = nc.dram_tensor(f"rs_e_in_{rep}", [NCH, 3, R, HID], BF16)
    rs_e_out = nc.dram_tensor(f"rs_e_out_{rep}", [3, R, HID], BF16)

    idx_i32, wx, wy = _roi_prep_ens(nc, pools, rois_t, gy_t, grid_t)
    get_w2(2, gate=rois_my[0:1, 0:4])
    w1e = []
    for h in range(3):
        t = pools["w1e"].tile([128, KB, HID], F8, tag=f"w1e{h}", name=f"w1e{h}")
        # gate the load on the final rois so it cannot steal the DMA pipe
        # from the stage phases
        nc.gpsimd.tensor_copy(t[0:1, 0:1, 0:4], rois_my[0:1, 0:4])
        nc.sync.dma_start(
            t[:], aps["w1p8"][:].rearrange("h (k p) n -> h p k n", p=128)[h])
        w1e.append(t)

    for c in range(NCH):
        xt = _pool_batch(nc, pools, feats_ap,
                         idx_i32[:, c, :, :].rearrange("p u y -> p (u y)"),
                         wy[:, c, 0:1],
                         lambda u: wx[:, c:c + 1, u:u + 1], ident_b)
        for h in range(3):
            psums = [pools["pfc"].tile([R, 512], F32, space="PSUM",
                                       tag=f"pf{j}", name=f"pf{j}")
                     for j in range(2)]
            _dr_matmuls(nc, psums, xt, w1e[h][:], start=True)
            _bias_stop(nc, psums, b1_t[0:1, h, :], eighth_b)
            sb = pools["rsio"].tile([R, HID], BF16, tag="rsiob", name="rsiob")
            nc.scalar.copy(sb[:, 0:512], psums[0][:])
            nc.vector.tensor_copy(sb[:, 512:1024], psums[1][:])
            nc.sync.dma_start(rs_e_in[:][c, h], sb[:])

    nc.gpsimd.collective_compute(
        "ReduceScatter", Alu.add, replica_groups=GROUPS,
        ins=[rs_e_in[:].opt()], outs=[rs_e_out[:].opt()])

    acc = pools["h"].tile([R, NCLS], F32, tag="acc", name="acc")
    for h in range(3):
        hs = pools["rsio"].tile([R, HID], BF16, tag="rsiob", name="hse")
        nc.sync.dma_start(hs[:], rs_e_out[:][h])
        h2T = head_tail(hs[:], h)
        logits = _fc_small(nc, pools, h2T, cls_t, bcls_t, h, NCLS, ones_b)
        p = _softmax(nc, pools, logits, str(h))
        if h == 0:
            nc.vector.tensor_copy(acc[:], p[:])
        else:
            nc.vector.tensor_tensor(acc[:], acc[:], p[:], op=Alu.add)

    outp = pools["h"].tile([R, NCLS], F32, tag="outp", name="outp")
    nc.vector.tensor_scalar(outp[:], acc[:], 1.0 / 3.0, None, op0=Alu.mult)
    nc.sync.dma_start(aps["out"][:], outp[:])


# ---------------------------------------------------------------------------
# host side
# ---------------------------------------------------------------------------

_CACHE: dict = {}


def build_program(reps: int = 1):
    nc = bacc.Bacc("TRN2", target_bir_lowering=False, debug=False,
                   num_devices=N_CORES)
    aps = {
        "feats_b": nc.dram_tensor("feats_b", [FEAT_ROWS, C], BF16,
                                  kind="ExternalInput").ap(),
        "rois_my": nc.dram_tensor("rois_my", [R, 4], F32,
                                  kind="ExternalInput").ap(),
        "w1f8": nc.dram_tensor("w1f8", [2, POOL * NU * C, HID], F8,
                               kind="ExternalInput").ap(),
        "w1p8": nc.dram_tensor("w1p8", [3, NU * C, HID], F8,
                               kind="ExternalInput").ap(),
        "fc1_b": nc.dram_tensor("fc1_b", [1, 3, HID], BF16,
                                kind="ExternalInput").ap(),
        "fc1_b8": nc.dram_tensor("fc1_b8", [1, 3, HID], BF16,
                                 kind="ExternalInput").ap(),
        "fc2_w": nc.dram_tensor("fc2_w", [3, HID, HID], BF16,
                                kind="ExternalInput").ap(),
        "fc2_b": nc.dram_tensor("fc2_b", [1, 3, HID], BF16,
                                kind="ExternalInput").ap(),
        "cls_w": nc.dram_tensor("cls_w", [3, HID, NCLS], BF16,
                                kind="ExternalInput").ap(),
        "cls_b": nc.dram_tensor("cls_b", [1, 3, NCLS], BF16,
                                kind="ExternalInput").ap(),
        "reg_w": nc.dram_tensor("reg_w", [3, HID, 4], BF16,
                                kind="ExternalInput").ap(),
        "reg_b": nc.dram_tensor("reg_b", [1, 3, 4], BF16,
                                kind="ExternalInput").ap(),
        "gridall": nc.dram_tensor("gridall", [128, POOL], F32,
                                  kind="ExternalInput").ap(),
        "gridy_e": nc.dram_tensor("gridy_e", [128, 1], F32,
                                  kind="ExternalInput").ap(),
        "stds_c": nc.dram_tensor("stds_c", [128, 4], F32,
                                 kind="ExternalInput").ap(),
        "out": nc.dram_tensor("out", [R, NCLS], F32,
                              kind="ExternalOutput").ap(),
    }
    with tile.TileContext(nc) as tc:
        for rep in range(reps):
            with ExitStack() as ctx:
                build_kernel(ctx, tc, aps, rep)
    nc.compile()
    return nc


def make_in_maps(inputs: dict) -> list:
    import ml_dtypes
    f32 = lambda x: np.ascontiguousarray(np.asarray(x, dtype=np.float32))
    bf16 = lambda x: np.ascontiguousarray(
        np.asarray(x, dtype=np.float32).astype(ml_dtypes.bfloat16))
    fp8 = lambda x: np.ascontiguousarray(
        np.asarray(x, dtype=np.float32).astype(mybir.dt.np(F8)))
    feats = np.concatenate([
        f32(inputs["P2"]).reshape(-1, C), f32(inputs["P3"]).reshape(-1, C),
        f32(inputs["P4"]).reshape(-1, C), f32(inputs["P5"]).reshape(-1, C),
    ], axis=0)
    rois = f32(inputs["rois"])
    stds_c = np.broadcast_to(
        np.array([0.1, 0.1, 0.2, 0.2], dtype=np.float32), (128, 4)).copy()
    grid = ((np.arange(POOL, dtype=np.float32) + np.float32(0.5))
            / np.float32(POOL))
    fc1_w = f32(inputs["fc1_w"])
    shared = {
        "feats_b": bf16(feats),
        "w1f8": fp8(fc1_w[:2]),
        "fc1_b": bf16(inputs["fc1_b"])[None],
        "fc1_b8": bf16(inputs["fc1_b"] / 8.0)[None],
        "fc2_w": bf16(inputs["fc2_w"]), "fc2_b": bf16(inputs["fc2_b"])[None],
        "cls_w": bf16(inputs["cls_w"]), "cls_b": bf16(inputs["cls_b"])[None],
        "reg_w": bf16(inputs["reg_w"]), "reg_b": bf16(inputs["reg_b"])[None],
        "gridall": np.broadcast_to(grid, (128, POOL)).copy(),
        "stds_c": stds_c,
    }
    in_maps = []
    for c in range(N_CORES):
        m = dict(shared)
        m["rois_my"] = rois[c * R:(c + 1) * R]
        if c < POOL:
            m["w1p8"] = fp8(fc1_w[:, c * NU * C:(c + 1) * NU * C, :])
            gy = grid[c]
        else:
            m["w1p8"] = np.zeros((3, NU * C, HID), mybir.dt.np(F8))
            gy = grid[POOL - 1]
        m["gridy_e"] = np.full((128, 1), gy, np.float32)
        in_maps.append(m)
    return in_maps


def make_runner(nc):
    import jax
    from jax.sharding import Mesh, PartitionSpec, NamedSharding
    from jax.experimental.shard_map import shard_map
    from concourse import bass2jax

    bass2jax.install_neuronx_cc_hook()
    pname = nc.partition_id_tensor.name if nc.partition_id_tensor else None
    in_names, out_names, out_avals = [], [], []
    for alloc in nc.m.functions[0].allocations:
        if not isinstance(alloc, mybir.MemoryLocationSet):
            continue
        name = alloc.memorylocations[0].name
        if alloc.kind == "ExternalInput":
            if name != pname:
                in_names.append(name)
        elif alloc.kind == "ExternalOutput":
            out_names.append(name)
            out_avals.append(jax.core.ShapedArray(
                tuple(alloc.tensor_shape), mybir.dt.np(alloc.dtype)))
    n_outs = len(out_avals)
    names_full = list(in_names) + out_names + ([pname] if pname else [])
    PER_CORE = {"rois_my", "w1p8", "gridy_e"}

    def _body(*args):
        ops = list(args)
        if pname is not None:
            ops.append(bass2jax.partition_id_tensor())
        return tuple(bass2jax._bass_exec_p.bind(
            *ops, out_avals=tuple(out_avals), in_names=tuple(names_full),
            out_names=tuple(out_names), lowering_input_output_aliases=(),
            sim_require_finite=False, sim_require_nnan=False, nc=nc))

    devices = jax.devices()[:N_CORES]
    mesh = Mesh(np.asarray(devices), ("core",))
    P_ = PartitionSpec
    in_specs = tuple(P_("core") if nm in PER_CORE else P_()
                     for nm in in_names) + (P_("core"),) * n_outs
    sharded = jax.jit(
        shard_map(_body, mesh=mesh, in_specs=in_specs,
                  out_specs=(P_("core"),) * n_outs, check_rep=False),
        keep_unused=True)

    def _args(in_maps):
        args = []
        for nm in in_names:
            if nm in PER_CORE:
                args.append(np.concatenate([m[nm] for m in in_maps], axis=0))
            else:
                args.append(in_maps[0][nm])
        args += [np.zeros((N_CORES * a.shape[0], *a.shape[1:]), a.dtype)
                 for a in out_avals]
        return args

    def prepare(in_maps):
        args = _args(in_maps)
        shards = [NamedSharding(mesh, s) for s in in_specs]
        return [jax.device_put(a, s) for a, s in zip(args, shards)]

    def run_dev(dev_args):
        out = sharded(*dev_args)
        jax.block_until_ready(out)
        return np.asarray(out[0])

    def run(in_maps):
        out = sharded(*_args(in_maps))
        jax.block_until_ready(out)
        return np.asarray(out[0])

    run.prepare = prepare
    run.run_dev = run_dev
    return run


def run_sim(nc, in_maps):
    """Execute the program on MultiCoreSim (cost-model timing + outputs)."""
    from concourse.bass_interp import MultiCoreSim
    sim = MultiCoreSim(nc, num_cores=N_CORES, require_finite=False,
                       require_nnan=False)
    for core_id in range(N_CORES):
        core = sim.cores[core_id]
        for name, val in in_maps[core_id].items():
            view = core.tensor(name)
            view[:] = np.asarray(val).reshape(view.shape)
    sim.simulate()
    out = np.concatenate(
        [np.asarray(sim.cores[c].tensor("out")) for c in range(N_CORES)],
        axis=0)
    return out, int(sim.global_time)


def kernel(**inputs) -> np.ndarray:
    if "nc" not in _CACHE:
        _CACHE["nc"] = build_program()
    in_maps = make_in_maps(inputs)
    out, hw_ns = run_sim(_CACHE["nc"], in_maps)
    _CACHE["hw_ns"] = hw_ns
    return out.astype(np.float32)
